# revision 27
# baseline (speedup 1.0000x reference)
"""GQA attention (16 q heads / 4 kv heads, HD=128, S=4096, D=2048) with RoPE,
causal mask, log-gate on kv positions, softmax, and output projection —
distributed over 8 NeuronCores.

Sharding: head-parallel. Core c computes q heads {2c, 2c+1} and kv head c//2.
Wq/Wk/Wv split column-wise, Wo row-wise; each core produces a partial [S, D]
output; host sums the 8 partials.

Single merged pipeline on-device (v3):
 - Attention computed transposed (scores^T [j, i], j = keys on partitions).
 - log(gate) - shift applied as the exp activation's per-partition bias, so
   denominators are plain sums of ex: accumulated on DVE in fp16 (2x mode),
   partition-reduced on the (otherwise idle) GpSimd engine, reciprocal via
   reciprocal_approx_fast. No M=1 rowsum matmuls.
 - Both heads share each scores PSUM tile ([128, 2, 256]) so one activation
   instruction computes exp for both heads of a key block (one bias column).
 - Diagonal key blocks restricted to their valid i-range; a single [128,128]
   triangular mask tile handles the diagonal itself. Upper-triangle blocks
   skipped entirely.
 - Projections for chunk nb+1 and Wo/output evacuation for chunk nb-1 are
   emitted as fillers inside the attention slot stream, keeping the PE
   continuously busy (p-state ramp to 2.4 GHz) and overlapping every engine.
"""

import math
from contextlib import ExitStack

import numpy as np

import concourse.bass as bass
import concourse.mybir as mybir
import concourse.tile as tile
from concourse import bacc, bass_isa
from concourse._compat import with_exitstack
from concourse.bass import ds
from concourse.bass_utils import run_bass_kernel_spmd
from concourse.masks import make_identity

P = 128
F = 512            # q-chunk per unit
HF = 256           # i-half per scores slot (1 PSUM bank for both heads)
S = 4096
D = 2048
HD = 128
KO = D // P        # 16 k-chunks for the projections
NB = S // F        # 8 sequence chunks
NJB = S // P       # 32 key blocks
F32 = mybir.dt.float32
BF16 = mybir.dt.bfloat16
FP16 = mybir.dt.float16
MULT = mybir.AluOpType.mult
ADD = mybir.AluOpType.add
EXPF = mybir.ActivationFunctionType.Exp


@with_exitstack
def _body(ctx: ExitStack, tc: tile.TileContext, io: dict):
    nc = tc.nc

    persist = ctx.enter_context(tc.tile_pool(name="persist", bufs=1))
    qT = persist.tile([P, 2, S], BF16, tag="qT")        # [d, h, i]
    kT = persist.tile([P, S], BF16, tag="kT")           # [d, j]
    vv = persist.tile([P, NJB, HD], FP16, tag="vv")     # [j, jb, d]
    attnT = persist.tile([P, 2, S], BF16, tag="attnT")  # [d, h, i] normalized
    loggate = persist.tile([P, NJB], F32, tag="lg")     # log(g)+shift [j, jb]
    tri = persist.tile([P, P], F32, tag="tri")          # 0 / -1e30 triangle
    ident = persist.tile([P, P], BF16, tag="ident")

    ones = persist.tile([P, 1], FP16, tag="ones")
    wpool = ctx.enter_context(tc.tile_pool(name="wpool", bufs=1))
    wq = wpool.tile([P, KO, 2 * HD], BF16, tag="wq")
    wq_r = io["wq"].rearrange("(ko p) m -> p ko m", p=P)
    for g in range(8):  # split so the first projection matmul starts sooner
        nc.sync.dma_start(wq[:, ds(g * 2, 2), :], wq_r[:, ds(g * 2, 2), :])
    nc.sync.dma_start(loggate[:], io["loggate"])
    nc.sync.dma_start(tri[:], io["tri"])
    make_identity(nc, ident[:])
    nc.vector.memset(ones[:], 1.0)
    wk = wpool.tile([P, KO, HD], BF16, tag="wk")
    wk_r = io["wk"].rearrange("(ko p) m -> p ko m", p=P)
    wv = wpool.tile([P, KO, HD], BF16, tag="wv")
    wv_r = io["wv"].rearrange("(ko p) m -> p ko m", p=P)
    for h in range(2):
        nc.sync.dma_start(wk[:, ds(h * 8, 8), :], wk_r[:, ds(h * 8, 8), :])
        nc.sync.dma_start(wv[:, ds(h * 8, 8), :], wv_r[:, ds(h * 8, 8), :])
    wo = wpool.tile([P, 2, D], BF16, tag="wo")

    xt_r = io["xt"].rearrange("(ko p) s -> p ko s", p=P)  # [128, 16, 4096]

    xt_pool = ctx.enter_context(tc.tile_pool(name="xt", bufs=12))
    tab_pool = ctx.enter_context(tc.tile_pool(name="tab", bufs=4))
    rope_pool = ctx.enter_context(tc.tile_pool(name="rope", bufs=2))
    vt_pool = ctx.enter_context(tc.tile_pool(name="vt", bufs=2))
    ex_pool = ctx.enter_context(tc.tile_pool(name="ex", bufs=8))
    acc_pool = ctx.enter_context(tc.tile_pool(name="acc", bufs=2))
    r2_pool = ctx.enter_context(tc.tile_pool(name="r2", bufs=2))
    rbc_pool = ctx.enter_context(tc.tile_pool(name="rbc", bufs=2))
    ob_pool = ctx.enter_context(tc.tile_pool(name="ob", bufs=3))
    psSc = ctx.enter_context(tc.tile_pool(name="psSc", bufs=2, space="PSUM"))
    psOut = ctx.enter_context(tc.tile_pool(name="psOut", bufs=1, space="PSUM"))
    psProj = ctx.enter_context(tc.tile_pool(name="psProj", bufs=1, space="PSUM"))
    psT = ctx.enter_context(tc.tile_pool(name="psT", bufs=1, space="PSUM"))
    psWo = ctx.enter_context(tc.tile_pool(name="psWo", bufs=2, space="PSUM"))

    # ------- projection machinery: per-chunk work as a list of emission
    # closures (filler items for the attention slot stream) -------
    def make_proj_items(c):
        """Emission closures computing qT/kT/vv for sequence chunk c."""
        sl = ds(c * F, F)
        state = {}

        def dma_x():
            xq = []
            for xi in range(4):
                xtile = xt_pool.tile([P, 4, F], BF16, tag="xt")
                for h in range(4):  # split across DMA queues (~21.6GB/s each)
                    nc.sync.dma_start(
                        xtile[:, ds(h, 1), :],
                        xt_r[:, ds(xi * 4 + h, 1), sl],
                    )
                xq.append(xtile)
            tabs = tab_pool.tile([P, 4, F], BF16, tag="tabs")
            for h in range(4):
                nc.sync.dma_start(
                    tabs[:, ds(h, 1), :], io["tabs"][:, ds(h, 1), sl]
                )
            state["xq"] = xq
            state["tabs"] = tabs

        def mm_group(w_sb, m0, g, key):
            def emit():
                if g == 0:
                    state[key] = psProj.tile([P, F], F32, tag="ps", name="ps")
                ps = state[key]
                for ko in range(g * 4, g * 4 + 4):
                    nc.tensor.matmul(
                        ps[:],
                        lhsT=w_sb[:, ko, ds(m0, P)],
                        rhs=state["xq"][ko // 4][:, ko % 4, :],
                        start=(ko == 0),
                        stop=(ko == KO - 1),
                    )
            return emit

        def rope_fin(key, trow, dest):
            def emit():
                ps = state[key]
                tabs = state["tabs"]
                tmp = rope_pool.tile([P, F], BF16, tag="tmp")
                nc.scalar.copy(tmp[:], ps[:])
                rot = rope_pool.tile([P, F], BF16, tag="rot")
                nc.sync.dma_start(rot[0:64, :], tmp[64:128, :])
                nc.sync.dma_start(rot[64:128, :], tmp[0:64, :])
                t1 = rope_pool.tile([P, F], BF16, tag="t1")
                nc.vector.tensor_tensor(t1[:], tmp[:], tabs[:, trow, :], op=MULT)
                r2 = rope_pool.tile([P, F], BF16, tag="r2t")
                nc.vector.tensor_tensor(r2[:], rot[:], tabs[:, trow + 1, :], op=MULT)
                nc.vector.tensor_tensor(dest, t1[:], r2[:], op=ADD)
            return emit

        def v_fin():
            ps = state["v"]
            vT = vt_pool.tile([P, F], BF16, tag="vT")
            nc.scalar.copy(vT[:], ps[:])
            for isub in range(4):
                pt = psT.tile([P, P], BF16, tag="pt")
                nc.tensor.transpose(pt[:], vT[:, ds(isub * P, P)], ident[:])
                nc.scalar.copy(vv[:, c * 4 + isub, :], pt[:])

        items = [dma_x]
        for g in range(4):
            items.append(mm_group(wq, 0, g, "q0"))
        items.append(rope_fin("q0", 0, qT[:, 0, sl]))
        for g in range(4):
            items.append(mm_group(wq, P, g, "q1"))
        items.append(rope_fin("q1", 0, qT[:, 1, sl]))
        for g in range(4):
            items.append(mm_group(wk, 0, g, "k"))
        items.append(rope_fin("k", 2, kT[:, sl]))
        for g in range(4):
            items.append(mm_group(wv, 0, g, "v"))
        items.append(v_fin)
        return items

    # ------- Wo output machinery -------
    wo_q = []  # pending (i2, e) output blocks
    wo_alt = [0]

    def emit_po():
        _, i2, e = wo_q.pop(0)
        po = psWo.tile([P, F], F32, tag="po")
        for hh in range(2):
            nc.tensor.matmul(
                po[:],
                lhsT=attnT[:, hh, ds(i2 * P, P)],
                rhs=wo[:, hh, ds(e * F, F)],
                start=(hh == 0),
                stop=(hh == 1),
            )
        ob = ob_pool.tile([P, F], F32, tag="ob")
        if wo_alt[0] % 2 == 0:
            nc.scalar.copy(ob[:], po[:])
        else:
            nc.vector.tensor_copy(ob[:], po[:])
        wo_alt[0] += 1
        nc.sync.dma_start(io["outp"][ds(i2 * P, P), ds(e * F, F)], ob[:])

    # ------- attention units -------
    proj_items = make_proj_items(0)
    for it in proj_items:
        it()  # chunk 0 projections up front
    nc.sync.dma_start(wo[:], io["wo"].rearrange("(h p) e -> p h e", p=P))
    proj_items = make_proj_items(1)
    proj_items.pop(0)()  # chunk-1 input DMA issued ahead of unit 0
    deferred = []  # tail closures of previous unit

    for nb in range(NB):
        out_h = [psOut.tile([P, F], F32, tag=f"o{h}", name=f"o{h}") for h in range(2)]
        ACC = acc_pool.tile([P, 2, F], FP16, tag="acc")
        pend = []  # out matmuls trail scores by 2 slots
        slot_idx = 0

        def emit_out(jb, ex, ih, i0, w, last):
            for hh in range(2):
                nc.tensor.matmul(
                    out_h[hh][:, ds(ih * HF + i0, w)],
                    lhsT=vv[:, jb, :],
                    rhs=ex[:, hh, ds(i0, w)],
                    start=(ih == 0 and jb == 0),
                    stop=last,
                )

        for ih in range(2):
            jmax = 4 * nb + 2 * ih + 2  # exclusive
            for jb in range(jmax):
                dp = jb - (4 * nb + 2 * ih)
                i0 = max(0, dp) * P
                w = HF - i0
                qoff = nb * F + ih * HF + i0
                # scores for both heads -> one PSUM bank
                sc = psSc.tile([P, 2, HF], F32, tag="sc")
                for hh in range(2):
                    nc.tensor.matmul(
                        sc[:, hh, ds(i0, w)],
                        lhsT=kT[:, ds(jb * P, P)],
                        rhs=qT[:, hh, ds(qoff, w)],
                        start=(hh == 0),
                        stop=(hh == 1),
                    )
                if dp >= 0:  # diagonal block: triangular mask
                    for hh in range(2):
                        nc.vector.tensor_tensor(
                            sc[:, hh, ds(i0, P)], sc[:, hh, ds(i0, P)], tri[:],
                            op=ADD,
                        )
                ex = ex_pool.tile([P, 2, HF], FP16, tag="ex")
                nc.scalar.activation(
                    ex[:, :, ds(i0, w)], sc[:, :, ds(i0, w)], EXPF,
                    bias=loggate[:, jb : jb + 1],
                )
                if len(pend) >= 2:
                    emit_out(*pend.pop(0), last=False)
                if jb == 0:
                    nc.vector.tensor_copy(ACC[:, :, ds(ih * HF, HF)], ex[:])
                else:
                    nc.vector.tensor_tensor(
                        ACC[:, :, ds(ih * HF + i0, w)],
                        ACC[:, :, ds(ih * HF + i0, w)],
                        ex[:, :, ds(i0, w)],
                        op=ADD,
                    )
                pend.append((jb, ex, ih, i0, w))
                # deferred tail of previous unit early in this unit
                if deferred and 1 <= slot_idx <= 3:
                    deferred.pop(0)()
                # fillers: Wo output blocks else next-chunk projection items;
                # extra fillers early in the unit cover the denominator chain
                # latency of the previous unit
                nfill = 2 if (slot_idx <= 3 or len(wo_q) > 24) else 1
                for _ in range(nfill):
                    if wo_q and (wo_q[0][0] <= nb - 2 or slot_idx >= 6):
                        emit_po()
                    elif proj_items:
                        proj_items.pop(0)()
                    else:
                        break
                slot_idx += 1
        while pend:
            emit_out(*pend.pop(0), last=(len(pend) == 0))

        # unit tail: denominators via two M=1 matmuls (partition-dim sums of
        # ACC), sharing the psProj bank; recip + broadcast + norm deferred
        den = psWo.tile([P, F], F32, tag="po", name="den")
        for hh in range(2):
            nc.tensor.matmul(
                den[hh * 64 : hh * 64 + 1, :],
                lhsT=ones[:],
                rhs=ACC[:, hh, :],
                start=True,
                stop=True,
            )
        r2s = [r2_pool.tile([1, F], F32, tag=f"r2{h}", name=f"r2{h}") for h in range(2)]
        rbc = rbc_pool.tile([P, 2, F], F32, tag="rbc")
        # recip + broadcast emitted here (not deferred): later po allocations
        # reuse den's PSUM bank, and the ring WAR only orders against readers
        # already emitted
        d1 = r2_pool.tile([1, F], F32, tag="d1", name="d1")
        nc.scalar.copy(d1[:], den[64:65, :])
        nc.vector.reciprocal_approx_fast(r2s[0][:], den[0:1, :])
        nc.vector.reciprocal_approx_fast(r2s[1][:], d1[:])
        for hh in range(2):
            nc.gpsimd.partition_broadcast(rbc[:, hh, :], r2s[hh][:])

        def make_norm(hh, nb=nb, r=rbc, o=out_h):
            def norm():
                nc.vector.tensor_tensor(
                    attnT[:, hh, ds(nb * F, F)], o[hh][:], r[:, hh, :], op=MULT
                )
            return norm

        wo_q.extend((nb, nb * 4 + i4, e) for i4 in range(4) for e in range(4))

        while deferred:
            deferred.pop(0)()
        deferred = [make_norm(0), make_norm(1)]

        # drain remaining projection items so chunk nb+1 is ready
        while proj_items:
            proj_items.pop(0)()
        if nb + 1 < NB:
            if nb + 2 < NB:
                proj_items = make_proj_items(nb + 2)
                proj_items.pop(0)()  # chunk nb+2 input DMA a full unit early
            else:
                proj_items = []

    while deferred:
        deferred.pop(0)()
    while wo_q:
        emit_po()


_NC_CACHE = None


def build_nc():
    global _NC_CACHE
    if _NC_CACHE is not None:
        return _NC_CACHE
    nc = bacc.Bacc("TRN2", target_bir_lowering=False, debug=False)
    io = {
        "xt": nc.dram_tensor("xt", [D, S], BF16, kind="ExternalInput").ap(),
        "wq": nc.dram_tensor("wq", [D, 2 * HD], BF16, kind="ExternalInput").ap(),
        "wk": nc.dram_tensor("wk", [D, HD], BF16, kind="ExternalInput").ap(),
        "wv": nc.dram_tensor("wv", [D, HD], BF16, kind="ExternalInput").ap(),
        "wo": nc.dram_tensor("wo", [2 * HD, D], BF16, kind="ExternalInput").ap(),
        "tabs": nc.dram_tensor("tabs", [P, 4, S], BF16, kind="ExternalInput").ap(),
        "loggate": nc.dram_tensor("loggate", [P, NJB], F32, kind="ExternalInput").ap(),
        "tri": nc.dram_tensor("tri", [P, P], F32, kind="ExternalInput").ap(),
        "outp": nc.dram_tensor("outp", [S, D], F32, kind="ExternalOutput").ap(),
    }
    with tile.TileContext(nc) as tc:
        _body(tc, io)
    nc.compile()
    _NC_CACHE = nc
    return nc


def make_in_maps(hidden_states, attention_mask, cos, sin, gate, Wq, Wk, Wv, Wo):
    import ml_dtypes
    bf16 = ml_dtypes.bfloat16
    X = np.asarray(hidden_states, np.float32).reshape(S, D)
    xt = np.ascontiguousarray(X.T.astype(bf16))
    cosT = np.ascontiguousarray(np.asarray(cos, np.float32).reshape(S, HD).T)
    sinT = np.ascontiguousarray(np.asarray(sin, np.float32).reshape(S, HD).T)
    sinTs = np.concatenate([-sinT[: HD // 2], sinT[HD // 2 :]], axis=0)
    sc = np.float32(1.0 / math.sqrt(HD))
    tabs = np.ascontiguousarray(
        np.stack([cosT * sc, sinTs * sc, cosT, sinTs], axis=1).astype(bf16)
    )
    # log gate with 2^-5 shift: keeps exp sums within fp16 range
    g = np.asarray(gate, np.float32).reshape(S) + np.float32(1e-8)
    lg = np.log(g).astype(np.float32) - np.float32(5.0 * math.log(2.0))
    loggate = np.ascontiguousarray(lg.reshape(NJB, P).T)
    jj = np.arange(P)[:, None]
    ii = np.arange(P)[None, :]
    tri = np.where(jj <= ii, np.float32(0), np.float32(-1e30))
    tri = np.ascontiguousarray(tri.astype(np.float32))

    Wq = np.asarray(Wq, np.float32)
    Wk = np.asarray(Wk, np.float32)
    Wv = np.asarray(Wv, np.float32)
    Wo = np.asarray(Wo, np.float32)

    in_maps = []
    for c in range(8):
        g128 = c // 2
        in_maps.append(
            {
                "xt": xt,
                "wq": np.ascontiguousarray(Wq[:, c * 256 : (c + 1) * 256].astype(bf16)),
                "wk": np.ascontiguousarray(Wk[:, g128 * HD : (g128 + 1) * HD].astype(bf16)),
                "wv": np.ascontiguousarray(Wv[:, g128 * HD : (g128 + 1) * HD].astype(bf16)),
                "wo": np.ascontiguousarray(Wo[c * 256 : (c + 1) * 256, :].astype(bf16)),
                "tabs": tabs,
                "loggate": loggate,
                "tri": tri,
            }
        )
    return in_maps


def kernel(hidden_states, attention_mask, cos, sin, gate, Wq, Wk, Wv, Wo,
           **kwargs):
    nc = build_nc()
    in_maps = make_in_maps(
        hidden_states, attention_mask, cos, sin, gate, Wq, Wk, Wv, Wo
    )
    res = run_bass_kernel_spmd(nc, in_maps, core_ids=list(range(8)), **kwargs)
    acc = res.results[0]["outp"].astype(np.float32).copy()
    for c in range(1, 8):
        acc += res.results[c]["outp"]
    out = acc.reshape(1, S, D)
    if kwargs:
        return out, res
    return out


# revision 28
# speedup vs baseline: 1.1273x; 1.1273x over previous
"""GQA attention (16 q heads / 4 kv heads, HD=128, S=4096, D=2048) with RoPE,
causal mask, log-gate on kv positions, softmax, and output projection —
distributed over 8 NeuronCores.

Sharding: head-parallel. Core c computes q heads {2c, 2c+1} and kv head c//2.
Wq/Wk/Wv split column-wise, Wo row-wise; each core produces a partial [S, D]
output; host sums the 8 partials.

Single merged pipeline on-device (v3):
 - Attention computed transposed (scores^T [j, i], j = keys on partitions).
 - log(gate) - shift applied as the exp activation's per-partition bias, so
   denominators are plain sums of ex: accumulated on DVE in fp16 (2x mode),
   partition-reduced on the (otherwise idle) GpSimd engine, reciprocal via
   reciprocal_approx_fast. No M=1 rowsum matmuls.
 - Both heads share each scores PSUM tile ([128, 2, 256]) so one activation
   instruction computes exp for both heads of a key block (one bias column).
 - Diagonal key blocks restricted to their valid i-range; a single [128,128]
   triangular mask tile handles the diagonal itself. Upper-triangle blocks
   skipped entirely.
 - Projections for chunk nb+1 and Wo/output evacuation for chunk nb-1 are
   emitted as fillers inside the attention slot stream, keeping the PE
   continuously busy (p-state ramp to 2.4 GHz) and overlapping every engine.
"""

import math
from contextlib import ExitStack

import numpy as np

import concourse.bass as bass
import concourse.mybir as mybir
import concourse.tile as tile
from concourse import bacc, bass_isa
from concourse._compat import with_exitstack
from concourse.bass import ds
from concourse.bass_utils import run_bass_kernel_spmd
from concourse.masks import make_identity

P = 128
F = 512            # q-chunk per unit
HF = 256           # i-half per scores slot (1 PSUM bank for both heads)
S = 4096
D = 2048
HD = 128
KO = D // P        # 16 k-chunks for the projections
NB = S // F        # 8 sequence chunks
NJB = S // P       # 32 key blocks
F32 = mybir.dt.float32
BF16 = mybir.dt.bfloat16
FP16 = mybir.dt.float16
MULT = mybir.AluOpType.mult
ADD = mybir.AluOpType.add
EXPF = mybir.ActivationFunctionType.Exp


@with_exitstack
def _body(ctx: ExitStack, tc: tile.TileContext, io: dict):
    nc = tc.nc

    persist = ctx.enter_context(tc.tile_pool(name="persist", bufs=1))
    qT = persist.tile([P, 2, S], BF16, tag="qT")        # [d, h, i]
    kT = persist.tile([P, S], BF16, tag="kT")           # [d, j]
    vv = persist.tile([P, NJB, HD], FP16, tag="vv")     # [j, jb, d]
    attnT = persist.tile([P, 2, S], BF16, tag="attnT")  # [d, h, i] normalized
    loggate = persist.tile([P, NJB], F32, tag="lg")     # log(g)+shift [j, jb]
    tri = persist.tile([P, P], F32, tag="tri")          # 0 / -1e30 triangle
    ident = persist.tile([P, P], BF16, tag="ident")

    ones = persist.tile([P, 1], FP16, tag="ones")
    wpool = ctx.enter_context(tc.tile_pool(name="wpool", bufs=1))
    wq = wpool.tile([P, KO, 2 * HD], BF16, tag="wq")
    wq_r = io["wq"].rearrange("(ko p) m -> p ko m", p=P)
    for g in range(8):  # split so the first projection matmul starts sooner
        nc.sync.dma_start(wq[:, ds(g * 2, 2), :], wq_r[:, ds(g * 2, 2), :])
    nc.sync.dma_start(loggate[:], io["loggate"])
    nc.sync.dma_start(tri[:], io["tri"])
    make_identity(nc, ident[:])
    nc.vector.memset(ones[:], 1.0)
    wk = wpool.tile([P, KO, HD], BF16, tag="wk")
    wk_r = io["wk"].rearrange("(ko p) m -> p ko m", p=P)
    wv = wpool.tile([P, KO, HD], BF16, tag="wv")
    wv_r = io["wv"].rearrange("(ko p) m -> p ko m", p=P)
    for h in range(2):
        nc.sync.dma_start(wk[:, ds(h * 8, 8), :], wk_r[:, ds(h * 8, 8), :])
        nc.sync.dma_start(wv[:, ds(h * 8, 8), :], wv_r[:, ds(h * 8, 8), :])
    wo = wpool.tile([P, 2, D], BF16, tag="wo")

    xt_r = io["xt"].rearrange("(ko p) s -> p ko s", p=P)  # [128, 16, 4096]

    xt_pool = ctx.enter_context(tc.tile_pool(name="xt", bufs=12))
    tab_pool = ctx.enter_context(tc.tile_pool(name="tab", bufs=4))
    rope_pool = ctx.enter_context(tc.tile_pool(name="rope", bufs=2))
    vt_pool = ctx.enter_context(tc.tile_pool(name="vt", bufs=2))
    ex_pool = ctx.enter_context(tc.tile_pool(name="ex", bufs=6))
    acc_pool = ctx.enter_context(tc.tile_pool(name="acc", bufs=2))
    r2_pool = ctx.enter_context(tc.tile_pool(name="r2", bufs=2))
    rbc_pool = ctx.enter_context(tc.tile_pool(name="rbc", bufs=2))
    ob_pool = ctx.enter_context(tc.tile_pool(name="ob", bufs=3))
    psSc = ctx.enter_context(tc.tile_pool(name="psSc", bufs=2, space="PSUM"))
    psOut = ctx.enter_context(tc.tile_pool(name="psOut", bufs=1, space="PSUM"))
    psProj = ctx.enter_context(tc.tile_pool(name="psProj", bufs=1, space="PSUM"))
    psT = ctx.enter_context(tc.tile_pool(name="psT", bufs=1, space="PSUM"))
    psWo = ctx.enter_context(tc.tile_pool(name="psWo", bufs=2, space="PSUM"))

    # ------- projection machinery: per-chunk work as a list of emission
    # closures (filler items for the attention slot stream) -------
    def make_proj_items(c):
        """Emission closures computing qT/kT/vv for sequence chunk c."""
        sl = ds(c * F, F)
        state = {}

        def dma_x():
            xq = []
            for xi in range(4):
                xtile = xt_pool.tile([P, 4, F], BF16, tag="xt")
                for h in range(2):  # split across DMA queues (~21.6GB/s each)
                    nc.sync.dma_start(
                        xtile[:, ds(h * 2, 2), :],
                        xt_r[:, ds(xi * 4 + h * 2, 2), sl],
                    )
                xq.append(xtile)
            tabs = tab_pool.tile([P, 4, F], BF16, tag="tabs")
            for h in range(2):
                nc.sync.dma_start(
                    tabs[:, ds(h * 2, 2), :], io["tabs"][:, ds(h * 2, 2), sl]
                )
            state["xq"] = xq
            state["tabs"] = tabs

        def mm_group(w_sb, m0, g, key):
            def emit():
                if g == 0:
                    state[key] = psProj.tile([P, F], F32, tag="ps", name="ps")
                ps = state[key]
                for ko in range(g * 4, g * 4 + 4):
                    nc.tensor.matmul(
                        ps[:],
                        lhsT=w_sb[:, ko, ds(m0, P)],
                        rhs=state["xq"][ko // 4][:, ko % 4, :],
                        start=(ko == 0),
                        stop=(ko == KO - 1),
                    )
            return emit

        def rope_fin(key, trow, dest):
            def emit():
                ps = state[key]
                tabs = state["tabs"]
                tmp = rope_pool.tile([P, F], BF16, tag="tmp")
                nc.scalar.copy(tmp[:], ps[:])
                rot = rope_pool.tile([P, F], BF16, tag="rot")
                nc.sync.dma_start(rot[0:64, :], tmp[64:128, :])
                nc.sync.dma_start(rot[64:128, :], tmp[0:64, :])
                t1 = rope_pool.tile([P, F], BF16, tag="t1")
                nc.vector.tensor_tensor(t1[:], tmp[:], tabs[:, trow, :], op=MULT)
                r2 = rope_pool.tile([P, F], BF16, tag="r2t")
                nc.vector.tensor_tensor(r2[:], rot[:], tabs[:, trow + 1, :], op=MULT)
                nc.vector.tensor_tensor(dest, t1[:], r2[:], op=ADD)
            return emit

        def v_fin():
            ps = state["v"]
            vT = vt_pool.tile([P, F], BF16, tag="vT")
            nc.scalar.copy(vT[:], ps[:])
            for isub in range(4):
                pt = psT.tile([P, P], BF16, tag="pt")
                nc.tensor.transpose(pt[:], vT[:, ds(isub * P, P)], ident[:])
                nc.scalar.copy(vv[:, c * 4 + isub, :], pt[:])

        items = [dma_x]
        for g in range(4):
            items.append(mm_group(wq, 0, g, "q0"))
        items.append(rope_fin("q0", 0, qT[:, 0, sl]))
        for g in range(4):
            items.append(mm_group(wq, P, g, "q1"))
        items.append(rope_fin("q1", 0, qT[:, 1, sl]))
        for g in range(4):
            items.append(mm_group(wk, 0, g, "k"))
        items.append(rope_fin("k", 2, kT[:, sl]))
        for g in range(4):
            items.append(mm_group(wv, 0, g, "v"))
        items.append(v_fin)
        return items

    # ------- Wo output machinery -------
    wo_q = []  # pending (i2, e) output blocks
    wo_alt = [0]

    def emit_po():
        _, i2, e = wo_q.pop(0)
        po = psWo.tile([P, F], F32, tag="po")
        for hh in range(2):
            nc.tensor.matmul(
                po[:],
                lhsT=attnT[:, hh, ds(i2 * P, P)],
                rhs=wo[:, hh, ds(e * F, F)],
                start=(hh == 0),
                stop=(hh == 1),
            )
        ob = ob_pool.tile([P, F], F32, tag="ob")
        if wo_alt[0] % 2 == 0:
            nc.scalar.copy(ob[:], po[:])
        else:
            nc.vector.tensor_copy(ob[:], po[:])
        wo_alt[0] += 1
        nc.sync.dma_start(io["outp"][ds(i2 * P, P), ds(e * F, F)], ob[:])

    # ------- attention units -------
    proj_items = make_proj_items(0)
    for it in proj_items:
        it()  # chunk 0 projections up front
    nc.sync.dma_start(wo[:], io["wo"].rearrange("(h p) e -> p h e", p=P))
    proj_items = make_proj_items(1)
    proj_items.pop(0)()  # chunk-1 input DMA issued ahead of unit 0
    deferred = []  # tail closures of previous unit

    for nb in range(NB):
        out_h = [psOut.tile([P, F], F32, tag=f"o{h}", name=f"o{h}") for h in range(2)]
        ACC = acc_pool.tile([P, 2, F], FP16, tag="acc")
        pend = []  # out matmuls trail scores by 2 slots
        slot_idx = 0

        def emit_out(jb, ex, ih, i0, w, last):
            for hh in range(2):
                nc.tensor.matmul(
                    out_h[hh][:, ds(ih * HF + i0, w)],
                    lhsT=vv[:, jb, :],
                    rhs=ex[:, hh, ds(i0, w)],
                    start=(ih == 0 and jb == 0),
                    stop=last,
                )

        for ih in range(2):
            jmax = 4 * nb + 2 * ih + 2  # exclusive
            for jb in range(jmax):
                dp = jb - (4 * nb + 2 * ih)
                i0 = max(0, dp) * P
                w = HF - i0
                qoff = nb * F + ih * HF + i0
                # scores for both heads -> one PSUM bank
                sc = psSc.tile([P, 2, HF], F32, tag="sc")
                for hh in range(2):
                    nc.tensor.matmul(
                        sc[:, hh, ds(i0, w)],
                        lhsT=kT[:, ds(jb * P, P)],
                        rhs=qT[:, hh, ds(qoff, w)],
                        start=(hh == 0),
                        stop=(hh == 1),
                    )
                if dp >= 0:  # diagonal block: triangular mask
                    for hh in range(2):
                        nc.vector.tensor_tensor(
                            sc[:, hh, ds(i0, P)], sc[:, hh, ds(i0, P)], tri[:],
                            op=ADD,
                        )
                ex = ex_pool.tile([P, 2, HF], FP16, tag="ex")
                nc.scalar.activation(
                    ex[:, :, ds(i0, w)], sc[:, :, ds(i0, w)], EXPF,
                    bias=loggate[:, jb : jb + 1],
                )
                if len(pend) >= 2:
                    emit_out(*pend.pop(0), last=False)
                if jb == 0:
                    nc.vector.tensor_copy(ACC[:, :, ds(ih * HF, HF)], ex[:])
                else:
                    nc.vector.tensor_tensor(
                        ACC[:, :, ds(ih * HF + i0, w)],
                        ACC[:, :, ds(ih * HF + i0, w)],
                        ex[:, :, ds(i0, w)],
                        op=ADD,
                    )
                pend.append((jb, ex, ih, i0, w))
                # deferred tail of previous unit early in this unit
                if deferred and 1 <= slot_idx <= 3:
                    deferred.pop(0)()
                # fillers: Wo output blocks else next-chunk projection items;
                # extra fillers early in the unit cover the denominator chain
                # latency of the previous unit
                nfill = 2 if (slot_idx <= 3 or len(wo_q) > 24) else 1
                for _ in range(nfill):
                    if wo_q and (wo_q[0][0] <= nb - 2 or slot_idx >= 6):
                        emit_po()
                    elif proj_items:
                        proj_items.pop(0)()
                    else:
                        break
                slot_idx += 1
        while pend:
            emit_out(*pend.pop(0), last=(len(pend) == 0))

        # unit tail: denominators via two M=1 matmuls (partition-dim sums of
        # ACC), sharing the psProj bank; recip + broadcast + norm deferred
        den = psWo.tile([P, F], F32, tag="po", name="den")
        for hh in range(2):
            nc.tensor.matmul(
                den[hh * 64 : hh * 64 + 1, :],
                lhsT=ones[:],
                rhs=ACC[:, hh, :],
                start=True,
                stop=True,
            )
        r2s = [r2_pool.tile([1, F], F32, tag=f"r2{h}", name=f"r2{h}") for h in range(2)]
        rbc = rbc_pool.tile([P, 2, F], F32, tag="rbc")
        # recip + broadcast emitted here (not deferred): later po allocations
        # reuse den's PSUM bank, and the ring WAR only orders against readers
        # already emitted
        d1 = r2_pool.tile([1, F], F32, tag="d1", name="d1")
        nc.scalar.copy(d1[:], den[64:65, :])
        nc.vector.reciprocal_approx_fast(r2s[0][:], den[0:1, :])
        nc.vector.reciprocal_approx_fast(r2s[1][:], d1[:])
        for hh in range(2):
            nc.gpsimd.partition_broadcast(rbc[:, hh, :], r2s[hh][:])

        def make_norm(hh, nb=nb, r=rbc, o=out_h):
            def norm():
                nc.vector.tensor_tensor(
                    attnT[:, hh, ds(nb * F, F)], o[hh][:], r[:, hh, :], op=MULT
                )
            return norm

        wo_q.extend((nb, nb * 4 + i4, e) for i4 in range(4) for e in range(4))

        while deferred:
            deferred.pop(0)()
        deferred = [make_norm(0), make_norm(1)]

        # drain remaining projection items so chunk nb+1 is ready
        while proj_items:
            proj_items.pop(0)()
        if nb + 1 < NB:
            if nb + 2 < NB:
                proj_items = make_proj_items(nb + 2)
                proj_items.pop(0)()  # chunk nb+2 input DMA a full unit early
            else:
                proj_items = []

    while deferred:
        deferred.pop(0)()
    while wo_q:
        emit_po()


_NC_CACHE = None


def build_nc():
    global _NC_CACHE
    if _NC_CACHE is not None:
        return _NC_CACHE
    nc = bacc.Bacc("TRN2", target_bir_lowering=False, debug=False)
    io = {
        "xt": nc.dram_tensor("xt", [D, S], BF16, kind="ExternalInput").ap(),
        "wq": nc.dram_tensor("wq", [D, 2 * HD], BF16, kind="ExternalInput").ap(),
        "wk": nc.dram_tensor("wk", [D, HD], BF16, kind="ExternalInput").ap(),
        "wv": nc.dram_tensor("wv", [D, HD], BF16, kind="ExternalInput").ap(),
        "wo": nc.dram_tensor("wo", [2 * HD, D], BF16, kind="ExternalInput").ap(),
        "tabs": nc.dram_tensor("tabs", [P, 4, S], BF16, kind="ExternalInput").ap(),
        "loggate": nc.dram_tensor("loggate", [P, NJB], F32, kind="ExternalInput").ap(),
        "tri": nc.dram_tensor("tri", [P, P], F32, kind="ExternalInput").ap(),
        "outp": nc.dram_tensor("outp", [S, D], F32, kind="ExternalOutput").ap(),
    }
    with tile.TileContext(nc) as tc:
        _body(tc, io)
    nc.compile()
    _NC_CACHE = nc
    return nc


def make_in_maps(hidden_states, attention_mask, cos, sin, gate, Wq, Wk, Wv, Wo):
    import ml_dtypes
    bf16 = ml_dtypes.bfloat16
    X = np.asarray(hidden_states, np.float32).reshape(S, D)
    xt = np.ascontiguousarray(X.T.astype(bf16))
    cosT = np.ascontiguousarray(np.asarray(cos, np.float32).reshape(S, HD).T)
    sinT = np.ascontiguousarray(np.asarray(sin, np.float32).reshape(S, HD).T)
    sinTs = np.concatenate([-sinT[: HD // 2], sinT[HD // 2 :]], axis=0)
    sc = np.float32(1.0 / math.sqrt(HD))
    tabs = np.ascontiguousarray(
        np.stack([cosT * sc, sinTs * sc, cosT, sinTs], axis=1).astype(bf16)
    )
    # log gate with 2^-5 shift: keeps exp sums within fp16 range
    g = np.asarray(gate, np.float32).reshape(S) + np.float32(1e-8)
    lg = np.log(g).astype(np.float32) - np.float32(5.0 * math.log(2.0))
    loggate = np.ascontiguousarray(lg.reshape(NJB, P).T)
    jj = np.arange(P)[:, None]
    ii = np.arange(P)[None, :]
    tri = np.where(jj <= ii, np.float32(0), np.float32(-1e30))
    tri = np.ascontiguousarray(tri.astype(np.float32))

    Wq = np.asarray(Wq, np.float32)
    Wk = np.asarray(Wk, np.float32)
    Wv = np.asarray(Wv, np.float32)
    Wo = np.asarray(Wo, np.float32)

    in_maps = []
    for c in range(8):
        g128 = c // 2
        in_maps.append(
            {
                "xt": xt,
                "wq": np.ascontiguousarray(Wq[:, c * 256 : (c + 1) * 256].astype(bf16)),
                "wk": np.ascontiguousarray(Wk[:, g128 * HD : (g128 + 1) * HD].astype(bf16)),
                "wv": np.ascontiguousarray(Wv[:, g128 * HD : (g128 + 1) * HD].astype(bf16)),
                "wo": np.ascontiguousarray(Wo[c * 256 : (c + 1) * 256, :].astype(bf16)),
                "tabs": tabs,
                "loggate": loggate,
                "tri": tri,
            }
        )
    return in_maps


def kernel(hidden_states, attention_mask, cos, sin, gate, Wq, Wk, Wv, Wo,
           **kwargs):
    nc = build_nc()
    in_maps = make_in_maps(
        hidden_states, attention_mask, cos, sin, gate, Wq, Wk, Wv, Wo
    )
    res = run_bass_kernel_spmd(nc, in_maps, core_ids=list(range(8)), **kwargs)
    acc = res.results[0]["outp"].astype(np.float32).copy()
    for c in range(1, 8):
        acc += res.results[c]["outp"]
    out = acc.reshape(1, S, D)
    if kwargs:
        return out, res
    return out
